# revision 1
# baseline (speedup 1.0000x reference)
"""Trainium2 Bass kernel for nn_DiffLoss2 (BCE-with-logits loss + accuracy).

reference:
    t = one_hot(sender, 128) reshaped [B, 1024]
    loss  = mean(max(x,0) - x*t + log1p(exp(-|x|)))  # == mean(softplus(x) - x*t)
    preds = argmax over each 128-wide group
    acc   = mean(all(preds == sender, axis=1)); acc_or = mean(preds == sender)

Device strategy (pure data parallel over 8 cores, batch-sharded; per core
the [8192, 1024] shard is processed as 32 fat tiles [128p, 2048]):
  ACT pass 1:  e = exp(x) written DIRECTLY into PSUM — this doubles as the
               penalty-bank seed, eliminating the separate seed copy.
  ACT pass 2:  ln(e + 1) (PSUM src) with accum -> per-partition softplus
               row sums (the loss's softplus term), output discarded.
  PE:          accumulates 4096*[(vh-sh)^2 + (vl-sl)^2] onto the PSUM bank
               via 4 k=24 matmuls (v = 16*vh+vl nibble split). All products
               and partial sums are integers < 2^24, so the penalty is
               EXACTLY 0 at v = sender and >= 4096 > e^8 elsewhere.
  DVE:         segmented min over PSUM  -> e^{x[sender]}  (bit-exact gather)
  GPSIMD:      pairwise max of group halves (2048 -> 1024 elems), halving
               the DVE segmented-max input.
  DVE:         segmented max over the pre-maxed halves -> m = max(x).
  ACT (once):  em = exp(m) over the tiny [P, 512] max buffer, so the match
               compare g >= em runs both sides through the SAME exp spline
               (exp is monotone; rounding is order-preserving).
  host:        loss = (sum(softplus) - sum(ln g)) / (B*1024) in float64
               match = (g >= em) -> acc, acc_or

The sender tensor never reaches the device: it is fully encoded in the lhs
matmul operands (O(B*A) host prep, like the sharding).
"""
import numpy as np

B, N_ATTR, N_VALS = 65536, 8, 128
N_CORES = 8
P = 128
BC = B // N_CORES          # rows per core: 8192
F = N_ATTR * N_VALS        # 1024
TF = 2048                  # tile free elems (2 rows of 1024)
NT = BC * F // (P * TF)    # fat tiles per core: 32
GPT = 2 * N_ATTR           # groups per tile row: 16
NMM = 4                    # matmuls per tile (512 cols each)
KPT = 6                    # k-rows per group (nibble-split quadratic)
KPM = 8 * KPT              # k-rows per matmul: 48 (8 groups/block)
NSLOT = 2                  # psum slots in flight (PSUM holds 2x 8KB)
BIG = 4096.0               # penalty scale; must exceed e^8 ~ 2981

_cache = {}


def _split_excess_waits(nc, cap=1):
    """This walrus build caps sync-wait commands per instruction; hoist
    excess waits onto InstNoOp carriers inserted before the instruction on
    the same engine (streams execute in order, so semantics hold)."""
    from concourse import mybir
    ctr = 0
    for f in nc.m.functions:
        for bb in f.blocks:
            new_list = []
            changed = False
            for ins in bb.instructions:
                si = ins.sync_info
                waits = list(si.on_wait) if si and si.on_wait else []
                if len(waits) > cap:
                    changed = True
                    for w in waits[:-cap]:
                        ctr += 1
                        nop = mybir.InstNoOp(name=f"WC-{ctr}", ins=[], outs=[])
                        nop.engine = ins.engine
                        nop.sync_info = mybir.SyncInfo(on_wait=[w], on_update=[])
                        new_list.append(nop)
                    ins.sync_info = mybir.SyncInfo(
                        on_wait=waits[-cap:], on_update=list(si.on_update or [])
                    )
                new_list.append(ins)
            if changed:
                bb.instructions = new_list


def _build_nc(R=1, deps=True, premax=False, bufs=(6, 3, 4)):
    import concourse.bass as bass
    import concourse.tile as tile
    from concourse import mybir
    import bass_rust as _br

    f32 = mybir.dt.float32
    bf16 = mybir.dt.bfloat16
    f16 = mybir.dt.float16
    nc = bass.Bass(trn_type="TRN2")
    x_d = nc.dram_tensor("x", [NT, P, TF], f16, kind="ExternalInput")
    lhs_d = nc.dram_tensor("lhs", [NT, NMM, KPM, P], bf16, kind="ExternalInput")
    rhs_d = nc.dram_tensor("rhs", [NMM, KPM, 512], bf16, kind="ExternalInput")
    em_d = nc.dram_tensor("em", [P, NT * 2 * GPT], f32, kind="ExternalOutput")
    g_d = nc.dram_tensor("g", [P, NT * 2 * GPT], f32, kind="ExternalOutput")
    sp_d = nc.dram_tensor("sp", [P, NT], f32, kind="ExternalOutput")

    with tile.TileContext(nc) as tc:
        with (
            tc.tile_pool(name="xp", bufs=bufs[0]) as xp,
            tc.tile_pool(name="pp", bufs=NSLOT, space="PSUM") as pp,
            tc.tile_pool(name="hp", bufs=bufs[1]) as hp,
            tc.tile_pool(name="lp", bufs=bufs[2]) as lp,
            tc.tile_pool(name="consts", bufs=1) as consts,
            tc.tile_pool(name="accum", bufs=1) as accum,
        ):
            rhs_t = consts.tile([KPM, NMM, 512], bf16)
            nc.sync.dma_start(out=rhs_t,
                              in_=rhs_d.rearrange("mm k c -> k mm c"))
            m_buf = accum.tile([P, NT * 2 * GPT], f16)
            g_buf = accum.tile([P, NT * 2 * GPT], f32)
            em_buf = accum.tile([P, NT * 2 * GPT], f32)
            sp_buf = accum.tile([P, NT], f32)
            ln_scr = consts.tile([P, TF], f32)   # discarded ln output

            # warm the ACT table set (Exp/Ln share natural_log_exp set)
            # before the pipeline starts
            warm = consts.tile([P, 2], f32)
            nc.vector.memset(warm, 0.0)
            warm2 = consts.tile([P, 2], f32)
            nc.scalar.activation(out=warm2, in_=warm,
                                 func=mybir.ActivationFunctionType.Exp)
            nc.scalar.activation(out=warm, in_=warm2,
                                 func=mybir.ActivationFunctionType.Ln,
                                 bias=1.0)

            prev_gmin = [None] * NSLOT
            pend = []

            def emit_min(item):
                t_, slot_, pw_, mmis_ = item
                gmin = nc.vector.tensor_reduce(
                    out=g_buf[:, t_ * 2 * GPT:(t_ + 1) * 2 * GPT],
                    in_=pw_.rearrange("p (h g v) -> p (h g) v",
                                      v=N_VALS // 2, g=GPT),
                    axis=mybir.AxisListType.X, op=mybir.AluOpType.min)
                if deps:
                    for mmi in mmis_:
                        _br.add_dep_helper(
                            gmin.ins, mmi.ins, sync=True,
                            reason="segmin after PE accumulate")
                prev_gmin[slot_] = gmin

            for r in range(R):
                for t in range(NT):
                    xt = xp.tile([P, TF], f16)
                    nc.sync.dma_start(out=xt, in_=x_d[t])
                    lhs_t = lp.tile([KPM, NMM, P], bf16)
                    nc.sync.dma_start(
                        out=lhs_t,
                        in_=lhs_d[t].rearrange("mm k p -> k mm p"))

                    # seed PSUM with exp(x) (also the softplus input)
                    pw = pp.tile([P, TF], f32)
                    ex = nc.scalar.activation(
                        out=pw, in_=xt,
                        func=mybir.ActivationFunctionType.Exp)
                    slot = (r * NT + t) % NSLOT
                    if deps and prev_gmin[slot] is not None:
                        _br.add_dep_helper(
                            ex.ins, prev_gmin[slot].ins, sync=True,
                            reason="psum slot reuse after segmin read")

                    # softplus row sums: ln(e + 1), accumulated
                    lni = nc.scalar.activation(
                        out=ln_scr, in_=pw,
                        func=mybir.ActivationFunctionType.Ln,
                        bias=1.0, accum_out=sp_buf[:, t:t + 1])

                    # segmented max m = max(x) over 128-wide groups. bf16
                    # tensor_tensor max runs at 2x (2x_1P uop exists), so
                    # two pairwise pre-max rounds (128 -> 32 per group)
                    # shrink the 1x-only tensor_reduce to 512 elems.
                    # columns are host-permuted to (v6, g, v5..v0), so the
                    # pairwise max over the v6 halves is two PLAIN contiguous
                    # [P, 1024] slices -> DVE 2x_1P mode engages.
                    if premax:
                        xh = hp.tile([P, TF // 2], f16)
                        nc.vector.tensor_max(
                            xh, xt[:, 0:TF // 2], xt[:, TF // 2:TF])
                        nc.vector.tensor_reduce(
                            out=m_buf[:, t * 2 * GPT:t * 2 * GPT + GPT],
                            in_=xh.rearrange("p (g v) -> p g v", v=N_VALS // 2),
                            axis=mybir.AxisListType.X,
                            op=mybir.AluOpType.max)
                        nc.vector.memset(
                            m_buf[:, t * 2 * GPT + GPT:(t + 1) * 2 * GPT],
                            -65000.0)
                    else:
                        # per-half partial maxes (h-major like segmin);
                        # host combines via max of the em halves
                        nc.vector.tensor_reduce(
                            out=m_buf[:, t * 2 * GPT:(t + 1) * 2 * GPT],
                            in_=xt.rearrange("p (h g v) -> p (h g) v",
                                             v=N_VALS // 2, g=GPT),
                            axis=mybir.AxisListType.X,
                            op=mybir.AluOpType.max)

                    # pw += 4096*[(vh-sh)^2 + (vl-sl)^2]  (exact; 0 at sender)
                    mmis = []
                    for mi in range(NMM):
                        mmi = nc.tensor.matmul(
                            out=pw[:, mi * 512:(mi + 1) * 512],
                            lhsT=lhs_t[:, mi, :], rhs=rhs_t[:, mi, :],
                            start=False, stop=True, skip_group_check=True)
                        if deps:
                            _br.add_dep_helper(
                                mmi.ins, lni.ins, sync=True,
                                reason="ln reads e-seed before PE accumulate")
                        mmis.append(mmi)

                    # segmented min -> e^{x[sender]}, bit-exact. Deferred
                    # by one tile: the min waits on this tile's matmuls, so
                    # emitting it after the NEXT tile's (independent) max
                    # path keeps the in-order DVE queue from idling.
                    pend.append((t, slot, pw, mmis))
                    if len(pend) == 2:
                        emit_min(pend.pop(0))

            while pend:
                emit_min(pend.pop(0))

            # em = exp(m): match compare in exp-space via the same spline
            nc.scalar.activation(out=em_buf, in_=m_buf,
                                 func=mybir.ActivationFunctionType.Exp)
            nc.sync.dma_start(out=em_d[:, :], in_=em_buf)
            nc.sync.dma_start(out=g_d[:, :], in_=g_buf)
            nc.sync.dma_start(out=sp_d[:, :], in_=sp_buf)

    _split_excess_waits(nc)
    return nc


def _get_nc():
    if "nc" not in _cache:
        _cache["nc"] = _build_nc()
    return _cache["nc"]


def _pack_operands(x, s):
    """Build per-core in_maps: x tiles + exact bf16 lhs rows + rhs."""
    import ml_dtypes
    bf = ml_dtypes.bfloat16

    # column permutation: new col (h, g, w) <- old col g*128 + h*64 + w
    h_, g_, w_ = np.meshgrid(np.arange(2), np.arange(GPT), np.arange(64),
                             indexing="ij")
    perm = (g_ * N_VALS + h_ * 64 + w_).reshape(-1)
    # rhs for one 512-col block: 8 groups x 64 cols, v = h*64 + w with h
    # fixed per block pair (blocks 0,1 -> h=0; 2,3 -> h=1)
    rhs = np.zeros((NMM, KPM, 512), np.float32)
    for mi in range(NMM):
        h = mi // 2
        for j in range(8):
            c = slice(j * 64, (j + 1) * 64)
            v = h * 64.0 + np.arange(64, dtype=np.float32)
            vh = np.floor_divide(v, 16.0)
            vl = v - 16.0 * vh
            rhs[mi, KPT * j + 0, c] = BIG * vh * vh
            rhs[mi, KPT * j + 1, c] = vh
            rhs[mi, KPT * j + 2, c] = 1.0
            rhs[mi, KPT * j + 3, c] = BIG * vl * vl
            rhs[mi, KPT * j + 4, c] = vl
            rhs[mi, KPT * j + 5, c] = 1.0
    rhs = rhs.astype(bf)

    in_maps = []
    for c in range(N_CORES):
        xs = np.ascontiguousarray(
            x[c * BC:(c + 1) * BC].reshape(NT, P, TF)[:, :, perm]
        ).astype(np.float16)
        sc = s[c * BC:(c + 1) * BC].astype(np.int64)
        # s_pack[p, t, b, a] = s[256t + 2p + b, a]
        sp_ = sc.reshape(NT, P, 2, N_ATTR).transpose(1, 0, 2, 3)
        sh = (sp_ >> 4).astype(np.float32)
        sl = (sp_ & 15).astype(np.float32)
        lhs = np.zeros((NT, NMM, KPM, P), np.float32)
        for mm in range(NMM):
            for j in range(8):
                gg = 8 * (mm % 2) + j
                b_, a_ = divmod(gg, N_ATTR)
                lhs[:, mm, KPT * j + 0, :] = 1.0
                lhs[:, mm, KPT * j + 1, :] = (-2.0 * BIG * sh[:, :, b_, a_]).T
                lhs[:, mm, KPT * j + 2, :] = (BIG * sh[:, :, b_, a_] ** 2).T
                lhs[:, mm, KPT * j + 3, :] = 1.0
                lhs[:, mm, KPT * j + 4, :] = (-2.0 * BIG * sl[:, :, b_, a_]).T
                lhs[:, mm, KPT * j + 5, :] = (BIG * sl[:, :, b_, a_] ** 2).T
        in_maps.append({"x": xs, "lhs": lhs.astype(bf), "rhs": rhs})
    return in_maps


def run_device(x, s, trace=False):
    from concourse.bass_utils import run_bass_kernel_spmd

    nc = _get_nc()
    x = np.ascontiguousarray(x, dtype=np.float32)
    s = np.asarray(s)
    in_maps = _pack_operands(x, s)
    if "warm" not in _cache:
        # throwaway first execution: cold-start (ACT table load etc.) can
        # race the PSUM seed on the very first run after model load
        run_bass_kernel_spmd(nc, in_maps, core_ids=list(range(N_CORES)))
        _cache["warm"] = True
    res = run_bass_kernel_spmd(nc, in_maps, core_ids=list(range(N_CORES)),
                               trace=trace)
    return res


def kernel(sender_input, receiver_output):
    x = np.asarray(receiver_output)
    s = np.asarray(sender_input)
    res = run_device(x, s)

    sp_total = 0.0
    lng_total = 0.0
    match_sum = 0
    allmatch_sum = 0
    for c in range(N_CORES):
        out = res.results[c]
        sp_total += out["sp"].astype(np.float64).sum()
        g2 = out["g"].reshape(P, NT, 2, GPT)
        g = np.minimum(g2[:, :, 0, :], g2[:, :, 1, :]).reshape(P, NT * GPT)
        em2 = out["em"].reshape(P, NT, 2, GPT)
        em = np.maximum(em2[:, :, 0, :], em2[:, :, 1, :]).reshape(P, NT * GPT)
        lng_total += np.log(g.astype(np.float64)).sum()
        match = g >= em
        # col t*16 + b*8 + a <-> row 256t + 2p + b, attr a
        match = match.reshape(P, NT, 2, N_ATTR)
        match_sum += match.sum()
        allmatch_sum += match.all(axis=3).sum()

    loss = (sp_total - lng_total) / (B * F)
    acc = allmatch_sum / B
    acc_or = match_sum / (B * N_ATTR)
    return (np.float32(loss), np.float32(acc), np.float32(acc_or))



# revision 10
# speedup vs baseline: 1.0006x; 1.0006x over previous
"""Trainium2 Bass kernel for nn_DiffLoss2 (BCE-with-logits loss + accuracy).

reference:
    t = one_hot(sender, 128) reshaped [B, 1024]
    loss  = mean(max(x,0) - x*t + log1p(exp(-|x|)))  # == mean(softplus(x) - x*t)
    preds = argmax over each 128-wide group
    acc   = mean(all(preds == sender, axis=1)); acc_or = mean(preds == sender)

Device strategy (pure data parallel over 8 cores, batch-sharded; per core
the [8192, 1024] fp16 shard is processed as 32 tiles [128p, 2048f]).

This part runs ACT at 1 elem/cycle/lane (1.2 GHz) and DVE at 1x/2x/4x
(0.96 GHz), so the kernel is engine-bound, not HBM-bound. The math is
decomposed so every element passes through exactly ONE ACT transcendental,
with the rest of the per-element work spread across DVE/PE/Pool:

  softplus(x) = relu(x) + ln(1+w),   w = exp(-|x|) in (0,1]
  sum(ln(1+w)) ~= c0*N + c1*sum(w) + c2*sum(w^2)   (minimax fit, |err|<0.004)
  -|x| = x - 2*relu(x)   (exact in fp16; avoids this build's broken abs op)

  DVE:  r2 = -2*relu(x) (one fused tensor_scalar: max 0, mult -2)
        mu = x + r2 = -|x| (tensor_tensor add, 2x)
        final segmented max 32->1 (tensor_reduce)
  ACT:  w = Exp(mu) -> bf16, accum -> sum(w)   [the ONE full transcendental]
  PE:   sum(relu) via block-ones column-sum matmuls on r2;
        sum(w^2) via 16 chunk self-matmuls (diagonal of w^T w, exact f32)
  Pool: first two segmax halving rounds (host-permuted columns make the
        group halves contiguous, which GpSimd requires)
  host: loss assembly; exact x[sender] gather (x and sender are host inputs,
        like the sharding itself); match = fp16(x_s) >= m -> acc, acc_or.

Host column permutation inside each group's 128 values:
  col'(b, a, v) = (v>>6)*1024 + ((v>>5)&1)*512 + ((v>>4)&1)*256
                  + b*128 + a*16 + (v&15)
so the three halving rounds pair (v6), (v5), (v4) — each a contiguous-half
pairing — leaving [P, 16 groups, 16] for the final reduce.
"""
import numpy as np

B, N_ATTR, N_VALS = 65536, 8, 128
N_CORES = 8
P = 128
BC = B // N_CORES          # rows per core: 8192
F = N_ATTR * N_VALS        # 1024
TF = 2048                  # tile free elems (2 rows of 1024)
NT = BC * F // (P * TF)    # tiles per core: 32
GPT = 2 * N_ATTR           # groups per tile: 16 (2 rows x 8 attrs)
MSPL = 960                 # mu columns computed on DVE; rest on Pool

# ln(1+w) ~= C0 + C1*w + C2*w^2 on w in (0,1], minimax |err| <= 0.0039
C0 = 0.00271826
C1 = 0.92790428
C2 = -0.24043291

_cache = {}


def _split_excess_waits(nc, cap=1):
    """This walrus build caps sync-wait commands per instruction; hoist
    excess waits onto InstNoOp carriers inserted before the instruction on
    the same engine (streams execute in order, so semantics hold)."""
    from concourse import mybir
    ctr = 0
    for f in nc.m.functions:
        for bb in f.blocks:
            new_list = []
            changed = False
            for ins in bb.instructions:
                si = ins.sync_info
                waits = list(si.on_wait) if si and si.on_wait else []
                if len(waits) > cap:
                    changed = True
                    for w in waits[:-cap]:
                        ctr += 1
                        nop = mybir.InstNoOp(name=f"WC-{ctr}", ins=[], outs=[])
                        nop.engine = ins.engine
                        nop.sync_info = mybir.SyncInfo(on_wait=[w], on_update=[])
                        new_list.append(nop)
                    ins.sync_info = mybir.SyncInfo(
                        on_wait=waits[-cap:], on_update=list(si.on_update or [])
                    )
                new_list.append(ins)
            if changed:
                bb.instructions = new_list


def _build_nc(R=1):
    import concourse.bass as bass
    import concourse.tile as tile
    from concourse import mybir

    f32 = mybir.dt.float32
    bf16 = mybir.dt.bfloat16
    f16 = mybir.dt.float16
    nc = bass.Bass(trn_type="TRN2")
    x_d = nc.dram_tensor("x", [NT, P, TF], f16, kind="ExternalInput")
    wq_d = nc.dram_tensor("wq", [P, 4], f16, kind="ExternalInput")
    m_d = nc.dram_tensor("m", [P, NT * GPT], f16, kind="ExternalOutput")
    sw_d = nc.dram_tensor("sw", [P, NT], f32, kind="ExternalOutput")
    cs_d = nc.dram_tensor("cs", [4, 512], f32, kind="ExternalOutput")
    w2_d = nc.dram_tensor("w2", [P, TF], f32, kind="ExternalOutput")

    with tile.TileContext(nc) as tc:
        with (
            tc.tile_pool(name="xp", bufs=4) as xp,
            tc.tile_pool(name="rp", bufs=3) as rp,
            tc.tile_pool(name="mp", bufs=3) as mp,
            tc.tile_pool(name="wp", bufs=3) as wp,
            tc.tile_pool(name="hp", bufs=3) as hp,
            tc.tile_pool(name="h2p", bufs=3) as h2p,
            tc.tile_pool(name="ppw", bufs=1, space="PSUM") as ppw,
            tc.tile_pool(name="ppc", bufs=1, space="PSUM") as ppc,
            tc.tile_pool(name="consts", bufs=1) as consts,
            tc.tile_pool(name="accum", bufs=1) as accum,
        ):
            wq_t = consts.tile([P, 4], f16)
            nc.sync.dma_start(out=wq_t, in_=wq_d[:, :])
            m_buf = accum.tile([P, NT * GPT], f16)
            sw_buf = accum.tile([P, NT], f32)
            w2_psum = ppw.tile([P, TF], f32)
            cs_psum = ppc.tile([4, 512], f32)
            nc.vector.memset(w2_psum, 0.0)
            nc.vector.memset(cs_psum, 0.0)
            cs_buf = accum.tile([4, 512], f32)
            w2_buf = accum.tile([P, TF], f32)

            # warm the exp table before the pipeline starts
            warm = consts.tile([P, 2], f32)
            nc.vector.memset(warm, 0.0)
            warm2 = consts.tile([P, 2], f32)
            nc.scalar.activation(out=warm2, in_=warm,
                                 func=mybir.ActivationFunctionType.Exp)

            for r in range(R):
                for t in range(NT):
                    xt = xp.tile([P, TF], f16)
                    nc.sync.dma_start(out=xt, in_=x_d[t])

                    # r2 = -2*relu(x)  (fused: max 0 then mult -2)
                    r2 = rp.tile([P, TF], f16)
                    nc.vector.tensor_scalar(
                        out=r2, in0=xt, scalar1=0.0, scalar2=-2.0,
                        op0=mybir.AluOpType.max,
                        op1=mybir.AluOpType.mult)

                    # mu = x + r2 = -|x|  (exact in fp16); columns split
                    # between DVE (2x) and Pool to balance engine load
                    mu = mp.tile([P, TF], f16)
                    nc.vector.tensor_add(mu[:, 0:MSPL], xt[:, 0:MSPL],
                                         r2[:, 0:MSPL])
                    nc.gpsimd.tensor_add(mu[:, MSPL:TF], xt[:, MSPL:TF],
                                         r2[:, MSPL:TF])

                    # w = exp(mu) in bf16, accumulating sum(w)
                    wt_ = wp.tile([P, TF], bf16)
                    nc.scalar.activation(
                        out=wt_, in_=mu,
                        func=mybir.ActivationFunctionType.Exp,
                        accum_out=sw_buf[:, t:t + 1])

                    # segmax halving rounds on DVE 2x (contiguous halves
                    # by host permutation): 2048 -> 1024 -> 512 -> 256, then
                    # a [P,16,16] reduce
                    xh = hp.tile([P, TF // 2], f16)
                    nc.vector.tensor_max(xh, xt[:, 0:TF // 2],
                                         xt[:, TF // 2:TF])
                    xh2 = h2p.tile([P, TF // 4], f16)
                    nc.vector.tensor_max(xh2, xh[:, 0:TF // 4],
                                         xh[:, TF // 4:TF // 2])
                    xh3 = h2p.tile([P, TF // 8], f16)
                    nc.vector.tensor_max(xh3, xh2[:, 0:TF // 8],
                                         xh2[:, TF // 8:TF // 4])
                    nc.vector.tensor_reduce(
                        out=m_buf[:, t * GPT:(t + 1) * GPT],
                        in_=xh3.rearrange("p (g v) -> p g v", v=16),
                        axis=mybir.AxisListType.X, op=mybir.AluOpType.max)

                    # PE: sum(relu) column sums (all four 512-blocks into one
                    # [4,512] stripe) + sum(w^2) via chunk self-matmuls
                    for j in range(4):
                        nc.tensor.matmul(
                            out=cs_psum[:, :],
                            lhsT=wq_t,
                            rhs=r2[:, 512 * j:512 * (j + 1)],
                            start=False, stop=True,
                            skip_group_check=True)
                    for c in range(16):
                        wsl = wt_[:, 128 * c:128 * (c + 1)]
                        nc.tensor.matmul(
                            out=w2_psum[:, 128 * c:128 * (c + 1)],
                            lhsT=wsl, rhs=wsl,
                            start=False, stop=True,
                            skip_group_check=True)

            nc.vector.tensor_copy(cs_buf, cs_psum[:, :])
            nc.scalar.copy(w2_buf, w2_psum[:, :])
            nc.sync.dma_start(out=m_d[:, :], in_=m_buf)
            nc.sync.dma_start(out=sw_d[:, :], in_=sw_buf)
            nc.sync.dma_start(out=cs_d[:, :], in_=cs_buf)
            nc.sync.dma_start(out=w2_d[:, :], in_=w2_buf)

    _split_excess_waits(nc)
    return nc


def _get_nc():
    if "nc" not in _cache:
        _cache["nc"] = _build_nc()
    return _cache["nc"]


def _perm():
    # col'(b, a, v) = (v>>6)*1024 + ((v>>5)&1)*512 + ((v>>4)&1)*256
    #                 + b*128 + a*16 + (v&15)
    # returns inverse mapping: for each packed col', the original col
    b, a, v = np.meshgrid(np.arange(2), np.arange(N_ATTR), np.arange(N_VALS),
                          indexing="ij")
    colp = ((v >> 6) * 1024 + ((v >> 5) & 1) * 512 + ((v >> 4) & 1) * 256
            + b * 128 + a * 16 + (v & 15))
    orig = b * 1024 + a * 128 + v
    inv = np.empty(TF, np.int64)
    inv[colp.reshape(-1)] = orig.reshape(-1)
    return inv


def _pack_operands(x, s):
    """Per-core in_maps: fp16 permuted tile-reshaped x + block-ones weights."""
    inv = _cache.setdefault("perm", _perm())
    wq = np.zeros((P, 4), np.float16)
    for m in range(4):
        wq[m * 32:(m + 1) * 32, m] = 1.0
    in_maps = []
    for c in range(N_CORES):
        xc = np.ascontiguousarray(
            x[c * BC:(c + 1) * BC], dtype=np.float16).reshape(NT, P, TF)
        xs = np.ascontiguousarray(xc[:, :, inv])
        in_maps.append({"x": xs, "wq": wq})
    return in_maps


def run_device(x, s, trace=False):
    from concourse.bass_utils import run_bass_kernel_spmd

    nc = _get_nc()
    x = np.ascontiguousarray(x, dtype=np.float32)
    s = np.asarray(s)
    in_maps = _pack_operands(x, s)
    if "warm" not in _cache:
        # throwaway first execution: cold-start (ACT table load etc.)
        run_bass_kernel_spmd(nc, in_maps, core_ids=list(range(N_CORES)))
        _cache["warm"] = True
    res = run_bass_kernel_spmd(nc, in_maps, core_ids=list(range(N_CORES)),
                               trace=trace)
    return res


def kernel(sender_input, receiver_output):
    x = np.asarray(receiver_output)
    s = np.asarray(sender_input).astype(np.int64)
    res = run_device(x, s)

    x32 = np.ascontiguousarray(x, dtype=np.float32)
    Ntot = B * F
    Srelu = 0.0
    Sw = 0.0
    Sw2 = 0.0
    Sxs = 0.0
    match_sum = 0
    allmatch_sum = 0
    didx = np.arange(P)
    for c in range(N_CORES):
        out = res.results[c]
        Srelu += out["cs"].astype(np.float64).sum() / -2.0
        Sw += out["sw"].astype(np.float64).sum()
        # sum(w^2) = sum over chunks of trace(w_c^T w_c)
        w2p = out["w2"].astype(np.float64).reshape(P, 16, P)
        Sw2 += w2p[didx, :, didx].sum()

        xc = x32[c * BC:(c + 1) * BC].reshape(BC, N_ATTR, N_VALS)
        sc = s[c * BC:(c + 1) * BC]
        xs_exact = np.take_along_axis(
            xc.astype(np.float64), sc[..., None], axis=2)[..., 0]
        Sxs += xs_exact.sum()

        # m cols: t*16 + b*8 + a  <->  row 256t + 2p + b, attr a
        m_rows = (out["m"].reshape(P, NT, 2, N_ATTR)
                  .transpose(1, 0, 2, 3).reshape(BC, N_ATTR))
        xs16 = np.take_along_axis(
            xc.astype(np.float16), sc[..., None], axis=2)[..., 0]
        match = xs16 >= m_rows
        match_sum += match.sum()
        allmatch_sum += match.all(axis=1).sum()

    Ssp = Srelu + C0 * Ntot + C1 * Sw + C2 * Sw2
    loss = (Ssp - Sxs) / Ntot
    acc = allmatch_sum / B
    acc_or = match_sum / (B * N_ATTR)
    return (np.float32(loss), np.float32(acc), np.float32(acc_or))


# revision 11
# speedup vs baseline: 1.3513x; 1.3505x over previous
"""Trainium2 Bass kernel for nn_DiffLoss2 (BCE-with-logits loss + accuracy).

reference:
    t = one_hot(sender, 128) reshaped [B, 1024]
    loss  = mean(max(x,0) - x*t + log1p(exp(-|x|)))  # == mean(softplus(x) - x*t)
    preds = argmax over each 128-wide group
    acc   = mean(all(preds == sender, axis=1)); acc_or = mean(preds == sender)

Device strategy (pure data parallel over 8 cores, batch-sharded; per core
the [8192, 1024] fp16 shard is processed as 32 tiles [128p, 2048f]).

This part runs ACT at 1 elem/cycle/lane (1.2 GHz) and DVE at 1x/2x/4x
(0.96 GHz), so the kernel is engine-bound, not HBM-bound. The math is
decomposed so every element passes through exactly ONE ACT transcendental,
with the rest of the per-element work spread across DVE/PE/Pool:

  softplus(x) = relu(x) + ln(1+w),   w = exp(-|x|) in (0,1]
  sum(ln(1+w)) ~= c0*N + c1*sum(w) + c2*sum(w^2)   (minimax fit, |err|<0.004)
  -|x| = x - 2*relu(x)   (exact in fp16; avoids this build's broken abs op)

  DVE:  r2 = -2*relu(x) (one fused tensor_scalar: max 0, mult -2)
        mu = x + r2 = -|x| (tensor_tensor add, 2x)
        final segmented max 32->1 (tensor_reduce)
  ACT:  w = Exp(mu) -> bf16, accum -> sum(w)   [the ONE full transcendental]
  PE:   sum(relu) via block-ones column-sum matmuls on r2;
        sum(w^2) via 16 chunk self-matmuls (diagonal of w^T w, exact f32)
  Pool: first two segmax halving rounds (host-permuted columns make the
        group halves contiguous, which GpSimd requires)
  host: loss assembly; exact x[sender] gather (x and sender are host inputs,
        like the sharding itself); match = fp16(x_s) >= m -> acc, acc_or.

Host column permutation inside each group's 128 values:
  col'(b, a, v) = (v>>6)*1024 + ((v>>5)&1)*512 + ((v>>4)&1)*256
                  + b*128 + a*16 + (v&15)
so the three halving rounds pair (v6), (v5), (v4) — each a contiguous-half
pairing — leaving [P, 16 groups, 16] for the final reduce.
"""
import numpy as np

B, N_ATTR, N_VALS = 65536, 8, 128
N_CORES = 8
P = 128
BC = B // N_CORES          # rows per core: 8192
F = N_ATTR * N_VALS        # 1024
TF = 2048                  # tile free elems (2 rows of 1024)
NT = BC * F // (P * TF)    # tiles per core: 32
GPT = 2 * N_ATTR           # groups per tile: 16 (2 rows x 8 attrs)
MSPL = 960                 # mu columns computed on DVE; rest on Pool

# ln(1+w) ~= C0 + C1*w + C2*w^2 on w in (0,1], minimax |err| <= 0.0039
C0 = 0.00271826
C1 = 0.92790428
C2 = -0.24043291

_cache = {}


def _split_excess_waits(nc, cap=1):
    """This walrus build caps sync-wait commands per instruction; hoist
    excess waits onto InstNoOp carriers inserted before the instruction on
    the same engine (streams execute in order, so semantics hold)."""
    from concourse import mybir
    ctr = 0
    for f in nc.m.functions:
        for bb in f.blocks:
            new_list = []
            changed = False
            for ins in bb.instructions:
                si = ins.sync_info
                waits = list(si.on_wait) if si and si.on_wait else []
                if len(waits) > cap:
                    changed = True
                    for w in waits[:-cap]:
                        ctr += 1
                        nop = mybir.InstNoOp(name=f"WC-{ctr}", ins=[], outs=[])
                        nop.engine = ins.engine
                        nop.sync_info = mybir.SyncInfo(on_wait=[w], on_update=[])
                        new_list.append(nop)
                    ins.sync_info = mybir.SyncInfo(
                        on_wait=waits[-cap:], on_update=list(si.on_update or [])
                    )
                new_list.append(ins)
            if changed:
                bb.instructions = new_list


def _build_nc(R=1):
    import concourse.bass as bass
    import concourse.tile as tile
    from concourse import mybir

    f32 = mybir.dt.float32
    bf16 = mybir.dt.bfloat16
    f16 = mybir.dt.float16
    nc = bass.Bass(trn_type="TRN2")
    x_d = nc.dram_tensor("x", [NT, P, TF], f16, kind="ExternalInput")
    wq_d = nc.dram_tensor("wq", [P, 4], f16, kind="ExternalInput")
    m_d = nc.dram_tensor("m", [P, NT * GPT], f16, kind="ExternalOutput")
    sw_d = nc.dram_tensor("sw", [P, NT], f32, kind="ExternalOutput")
    cs_d = nc.dram_tensor("cs", [4, 512], f32, kind="ExternalOutput")
    w2_d = nc.dram_tensor("w2", [P, TF], f32, kind="ExternalOutput")

    with tile.TileContext(nc) as tc:
        with (
            tc.tile_pool(name="xp", bufs=6) as xp,
            tc.tile_pool(name="rp", bufs=3) as rp,
            tc.tile_pool(name="mp", bufs=3) as mp,
            tc.tile_pool(name="wp", bufs=3) as wp,
            tc.tile_pool(name="hp", bufs=3) as hp,
            tc.tile_pool(name="h2p", bufs=3) as h2p,
            tc.tile_pool(name="ppw", bufs=1, space="PSUM") as ppw,
            tc.tile_pool(name="ppc", bufs=1, space="PSUM") as ppc,
            tc.tile_pool(name="consts", bufs=1) as consts,
            tc.tile_pool(name="accum", bufs=1) as accum,
        ):
            wq_t = consts.tile([P, 4], f16)
            nc.sync.dma_start(out=wq_t, in_=wq_d[:, :])
            m_buf = accum.tile([P, NT * GPT], f16)
            sw_buf = accum.tile([P, NT], f32)
            w2_psum = ppw.tile([P, TF], f32)
            cs_psum = ppc.tile([4, 512], f32)
            nc.vector.memset(w2_psum, 0.0)
            nc.vector.memset(cs_psum, 0.0)
            cs_buf = accum.tile([4, 512], f32)
            w2_buf = accum.tile([P, TF], f32)

            # warm the exp table before the pipeline starts
            warm = consts.tile([P, 2], f32)
            nc.vector.memset(warm, 0.0)
            warm2 = consts.tile([P, 2], f32)
            nc.scalar.activation(out=warm2, in_=warm,
                                 func=mybir.ActivationFunctionType.Exp)

            for r in range(R):
                for t in range(NT):
                    xt = xp.tile([P, TF], f16)
                    nc.sync.dma_start(out=xt, in_=x_d[t])

                    # r2 = -2*relu(x)  (fused: max 0 then mult -2)
                    r2 = rp.tile([P, TF], f16)
                    nc.vector.tensor_scalar(
                        out=r2, in0=xt, scalar1=0.0, scalar2=-2.0,
                        op0=mybir.AluOpType.max,
                        op1=mybir.AluOpType.mult)

                    # mu = x + r2 = -|x|  (exact in fp16, DVE 2x; GpSimd
                    # measured ~0.35 efficiency, so no Pool split)
                    mu = mp.tile([P, TF], f16)
                    nc.vector.tensor_add(mu, xt, r2)

                    # w = exp(mu) in bf16, accumulating sum(w)
                    wt_ = wp.tile([P, TF], bf16)
                    nc.scalar.activation(
                        out=wt_, in_=mu,
                        func=mybir.ActivationFunctionType.Exp,
                        accum_out=sw_buf[:, t:t + 1])

                    # segmax halving rounds on DVE 2x (contiguous halves
                    # by host permutation): 2048 -> 1024 -> 512 -> 256, then
                    # a [P,16,16] reduce
                    xh = hp.tile([P, TF // 2], f16)
                    nc.vector.tensor_max(xh, xt[:, 0:TF // 2],
                                         xt[:, TF // 2:TF])
                    xh2 = h2p.tile([P, TF // 4], f16)
                    nc.vector.tensor_max(xh2, xh[:, 0:TF // 4],
                                         xh[:, TF // 4:TF // 2])
                    xh3 = h2p.tile([P, TF // 8], f16)
                    nc.vector.tensor_max(xh3, xh2[:, 0:TF // 8],
                                         xh2[:, TF // 8:TF // 4])
                    nc.vector.tensor_reduce(
                        out=m_buf[:, t * GPT:(t + 1) * GPT],
                        in_=xh3.rearrange("p (g v) -> p g v", v=16),
                        axis=mybir.AxisListType.X, op=mybir.AluOpType.max)

                    # PE: sum(relu) column sums (all four 512-blocks into one
                    # [4,512] stripe) + sum(w^2) via chunk self-matmuls
                    for j in range(4):
                        nc.tensor.matmul(
                            out=cs_psum[:, :],
                            lhsT=wq_t,
                            rhs=r2[:, 512 * j:512 * (j + 1)],
                            start=False, stop=True,
                            skip_group_check=True)
                    for c in range(16):
                        wsl = wt_[:, 128 * c:128 * (c + 1)]
                        nc.tensor.matmul(
                            out=w2_psum[:, 128 * c:128 * (c + 1)],
                            lhsT=wsl, rhs=wsl,
                            start=False, stop=True,
                            skip_group_check=True)

            nc.vector.tensor_copy(cs_buf, cs_psum[:, :])
            nc.scalar.copy(w2_buf, w2_psum[:, :])
            nc.sync.dma_start(out=m_d[:, :], in_=m_buf)
            nc.sync.dma_start(out=sw_d[:, :], in_=sw_buf)
            nc.sync.dma_start(out=cs_d[:, :], in_=cs_buf)
            nc.sync.dma_start(out=w2_d[:, :], in_=w2_buf)

    _split_excess_waits(nc)
    return nc


def _get_nc():
    if "nc" not in _cache:
        _cache["nc"] = _build_nc()
    return _cache["nc"]


def _perm():
    # col'(b, a, v) = (v>>6)*1024 + ((v>>5)&1)*512 + ((v>>4)&1)*256
    #                 + b*128 + a*16 + (v&15)
    # returns inverse mapping: for each packed col', the original col
    b, a, v = np.meshgrid(np.arange(2), np.arange(N_ATTR), np.arange(N_VALS),
                          indexing="ij")
    colp = ((v >> 6) * 1024 + ((v >> 5) & 1) * 512 + ((v >> 4) & 1) * 256
            + b * 128 + a * 16 + (v & 15))
    orig = b * 1024 + a * 128 + v
    inv = np.empty(TF, np.int64)
    inv[colp.reshape(-1)] = orig.reshape(-1)
    return inv


def _pack_operands(x, s):
    """Per-core in_maps: fp16 permuted tile-reshaped x + block-ones weights."""
    inv = _cache.setdefault("perm", _perm())
    wq = np.zeros((P, 4), np.float16)
    for m in range(4):
        wq[m * 32:(m + 1) * 32, m] = 1.0
    in_maps = []
    for c in range(N_CORES):
        xc = np.ascontiguousarray(
            x[c * BC:(c + 1) * BC], dtype=np.float16).reshape(NT, P, TF)
        xs = np.ascontiguousarray(xc[:, :, inv])
        in_maps.append({"x": xs, "wq": wq})
    return in_maps


def run_device(x, s, trace=False):
    from concourse.bass_utils import run_bass_kernel_spmd

    nc = _get_nc()
    x = np.ascontiguousarray(x, dtype=np.float32)
    s = np.asarray(s)
    in_maps = _pack_operands(x, s)
    if "warm" not in _cache:
        # throwaway first execution: cold-start (ACT table load etc.)
        run_bass_kernel_spmd(nc, in_maps, core_ids=list(range(N_CORES)))
        _cache["warm"] = True
    res = run_bass_kernel_spmd(nc, in_maps, core_ids=list(range(N_CORES)),
                               trace=trace)
    return res


def kernel(sender_input, receiver_output):
    x = np.asarray(receiver_output)
    s = np.asarray(sender_input).astype(np.int64)
    res = run_device(x, s)

    x32 = np.ascontiguousarray(x, dtype=np.float32)
    Ntot = B * F
    Srelu = 0.0
    Sw = 0.0
    Sw2 = 0.0
    Sxs = 0.0
    match_sum = 0
    allmatch_sum = 0
    didx = np.arange(P)
    for c in range(N_CORES):
        out = res.results[c]
        Srelu += out["cs"].astype(np.float64).sum() / -2.0
        Sw += out["sw"].astype(np.float64).sum()
        # sum(w^2) = sum over chunks of trace(w_c^T w_c)
        w2p = out["w2"].astype(np.float64).reshape(P, 16, P)
        Sw2 += w2p[didx, :, didx].sum()

        xc = x32[c * BC:(c + 1) * BC].reshape(BC, N_ATTR, N_VALS)
        sc = s[c * BC:(c + 1) * BC]
        xs_exact = np.take_along_axis(
            xc.astype(np.float64), sc[..., None], axis=2)[..., 0]
        Sxs += xs_exact.sum()

        # m cols: t*16 + b*8 + a  <->  row 256t + 2p + b, attr a
        m_rows = (out["m"].reshape(P, NT, 2, N_ATTR)
                  .transpose(1, 0, 2, 3).reshape(BC, N_ATTR))
        xs16 = np.take_along_axis(
            xc.astype(np.float16), sc[..., None], axis=2)[..., 0]
        match = xs16 >= m_rows
        match_sum += match.sum()
        allmatch_sum += match.all(axis=1).sum()

    Ssp = Srelu + C0 * Ntot + C1 * Sw + C2 * Sw2
    loss = (Ssp - Sxs) / Ntot
    acc = allmatch_sum / B
    acc_or = match_sum / (B * N_ATTR)
    return (np.float32(loss), np.float32(acc), np.float32(acc_or))
